# revision 2
# baseline (speedup 1.0000x reference)
"""BalanceLoss (BCE + OHEM top-k negatives) on 8 trn2 NeuronCores — v2.

Algorithm
---------
Host encodes the two {0,1} label tensors as one categorical bf16 tensor
    e = mask * (2*gt - 1)  in {-1, 0, +1}   (pos / ignore / neg label)
Per core (data-parallel shard of 1/8 of the elements, [128 x 12800]):
    y = pred_centered * e                   (tensor_tensor, Pool/DVE;
                                             host ships pred - 1/2)
    z = Ln(y + 1/2)                         (ScalarE)
      = ln(pred)   where e=+1   (positive, masked-in)
      = ln(1-pred) where e=-1   (negative)
      = ln(1/2)    where e= 0   (masked-out; exact, host-corrected)
    z tiles are float32r so the PE column-sum runs at 1 cycle/row without
    bf16 rounding bias.
    Sz  = sum(z)       PE ones-matmul column sums into PSUM banks 0-3
    Se  = sum(e)       PE ones-matmul column sums into PSUM banks 4-7
    sw  = sum(relu(e)) DVE tensor_scalar(max 0) + accum (4x bf16 rate)
Host merge (exact f64): sn = sw - Se, Sm = sw + sn,
    pos_loss+neg_loss_all = -(Sz - (N-Sm)*ln(1/2)_bf16)
OHEM top-k == all-negatives whenever k = min(sn, 3*sw) == sn (true for this
distribution); exact host fallback otherwise.

Scheduling: all pred/e tiles stay resident in SBUF (no buffer recycling), and
the program is emitted from an explicit token sequence so each engine queue
(SP/Act/Pool DMA+compute, DVE, PE) receives jobs in a hand-tuned order that
keeps the ScalarE Ln stream — the longest serial chain — fed without stalls.
"""

import os
import sys

import numpy as np

# ---------------------------------------------------------------- constants
FULL_SHAPE = (32, 1, 640, 640)
TOT = 32 * 640 * 640          # 13_107_200 elements
N_CORES = 8
PER_CORE = TOT // N_CORES     # 1_638_400
P = 128                       # SBUF partitions
W = PER_CORE // P             # 12_800 free-dim elements per partition
NEG_RATIO = 3.0
EPS = 1e-6

TILE_PLAN = (512, 1024, 1024, 1536, 2048, 2048, 2048, 2048, 512)
assert sum(TILE_PLAN) == W
NT = len(TILE_PLAN)

# y-engine per tile: 'd' = DVE, 'g' = Pool
Y_ENG = ("g", "g", "d", "d", "d", "d", "d", "g", "g")

# Emission order. Tokens:
#   P<t>@<q>   pred-tile DMA on queue q (s/a/g); Pa/Pb = first/second half
#   E<t>@<q>   e-tile DMA
#   Y<t>       y stt (engine from Y_ENG)
#   T<t>       sum(relu(e)) = pos_count tensor_scalar on DVE
#   M<t>       PE e-matmuls for tile t
#   A<t>       Ln act on ScalarE
#   ZM<t>      PE z-matmuls for tile t (needs A<t> first)
#   FE / FZ    psum folds (e / z streams; split over Pool+DVE)
#   STATS      final stats DMA on SP
# PE stream init: M3/ZM3 are emitted first on PE with start=True (tile 3 is
# full-width, so its write initializes all 512 cols of each psum group).
SCHEDULE = (
    "P0@s", "E0@a", "P1@g", "E1@a", "P2@s", "E2@g",
    "Y0", "T0", "A0",
    "Pa3@s", "Pb3@g", "E3@g",
    "Y1", "T1", "A1",
    "Pa4@s", "Pb4@g", "E4@s", "M4", "M0", "M1",
    "Y2", "T2", "A2",
    "Pa5@s", "Pb5@g", "E5@s", "M2", "M3",
    "Y3", "T3", "A3",
    "Pa6@s", "Pb6@g", "E6@s",
    "Y4", "T4", "A4", "ZM4", "ZM0", "ZM1", "ZM2", "ZM3",
    "Pa7@s", "Pb7@g", "Ea7@a", "Eb7@g", "M5",
    "Y5", "T5", "A5", "ZM5",
    "P8@s", "E8@s", "M6",
    "Y6", "T6", "A6", "ZM6",
    "M7", "M8",
    "Y7", "T7", "A7", "ZM7",
    "Y8", "T8", "A8", "ZM8",
    "FE", "FZ", "STATS",
)

_CONCOURSE_PATHS = ("/opt/trn_rl_repo", "/root/.axon_site/_ro/trn_rl_repo")


def _ensure_concourse():
    try:
        import concourse.bass  # noqa: F401
    except ImportError:
        for p in _CONCOURSE_PATHS:
            if os.path.isdir(p) and p not in sys.path:
                sys.path.insert(0, p)
        import concourse.bass  # noqa: F401


_NC_CACHE = {}

# ln(0.5) in fp32 — matches the device z value for masked-out elements
# (the Ln argument is exactly 0.5 there; z tiles are fp32(r)).
def _ln_half_bf16():
    return float(np.float32(np.log(np.float32(0.5))))


def _build_nc(plan=TILE_PLAN, y_eng=Y_ENG, schedule=SCHEDULE):
    key = (plan, y_eng, schedule)
    if key in _NC_CACHE:
        return _NC_CACHE[key]
    _ensure_concourse()
    import concourse.bacc as bacc
    import concourse.mybir as mybir
    import concourse.tile as tile

    f32 = mybir.dt.float32
    f32r = mybir.dt.float32r
    bf16 = mybir.dt.bfloat16
    Act = mybir.ActivationFunctionType
    Alu = mybir.AluOpType

    nt = len(plan)
    offs = [sum(plan[:i]) for i in range(nt)]
    last_m = max(i for i, tok in enumerate(schedule) if tok.startswith("M"))
    last_zm = max(i for i, tok in enumerate(schedule) if tok.startswith("ZM"))
    # stats: [0:nt]=sum(|e|) per tile, [nt:nt+4]=Se folds, [nt+4:nt+8]=Sz folds
    SCOLS = nt + 8

    nc = bacc.Bacc(None, target_bir_lowering=False)
    predD = nc.declare_dram_parameter("pred", [P, W], f32, isOutput=False)
    eD = nc.declare_dram_parameter("e", [P, W], bf16, isOutput=False)
    outD = nc.declare_dram_parameter("stats", [P, SCOLS], f32, isOutput=True)

    with tile.TileContext(nc) as tc:
        eng = {"s": nc.sync, "a": nc.scalar, "g": nc.gpsimd}
        with (
            tc.tile_pool(name="iop", bufs=1) as iop,
            tc.tile_pool(name="ioe", bufs=1) as ioe,
            tc.tile_pool(name="ytmp", bufs=3) as ypool,
            tc.tile_pool(name="ztmp", bufs=6) as zpool,
            tc.tile_pool(name="junk", bufs=2) as jpool,
            tc.tile_pool(name="accp", bufs=1) as acc_pool,
            tc.tile_pool(name="ps", bufs=1, space="PSUM") as ps_pool,
        ):
            acc = acc_pool.tile([P, SCOLS], f32)
            nc.vector.memset(acc[:], 0.0)
            ones = acc_pool.tile([P, 1], bf16)
            nc.gpsimd.memset(ones[:], 1.0)
            onesf = acc_pool.tile([P, 1], f32)
            nc.gpsimd.memset(onesf[:], 1.0)
            half = acc_pool.tile([P, 1], f32)
            nc.gpsimd.memset(half[:], 0.5)
            # 8 psum banks: z-stream groups 0-3 (cols 512c), e-stream 4-7.
            # The first-emitted matmul of each stream is a full-width tile
            # with start=True, initializing all 512 cols of each group.
            psz = ps_pool.tile([1, 2048], f32, tag="psz")
            pse = ps_pool.tile([1, 2048], f32, tag="pse")

            preds = [iop.tile([P, F], f32, tag=f"pred{t}", name=f"pred{t}")
                     for t, F in enumerate(plan)]
            es = [ioe.tile([P, F], bf16, tag=f"e{t}", name=f"e{t}")
                  for t, F in enumerate(plan)]

            first_m = [True]
            first_zm = [True]
            ycache = {}
            zcache = {}

            for ti, tok in enumerate(schedule):
                if "@" in tok:
                    q = tok[-1]
                    body = tok[: tok.index("@")]
                    if body[0] == "P":
                        half_sel = None
                        if body[1] in "ab":
                            half_sel, t = body[1], int(body[2:])
                        else:
                            t = int(body[1:])
                        F = plan[t]
                        lo, hi = 0, F
                        if half_sel == "a":
                            hi = F // 2
                        elif half_sel == "b":
                            lo = F // 2
                        sl = slice(offs[t] + lo, offs[t] + hi)
                        eng[q].dma_start(preds[t][:, lo:hi], predD[:, sl])
                    else:
                        half_sel = None
                        if body[1] in "ab":
                            half_sel, t = body[1], int(body[2:])
                        else:
                            t = int(body[1:])
                        F = plan[t]
                        lo, hi = 0, F
                        if half_sel == "a":
                            hi = F // 2
                        elif half_sel == "b":
                            lo = F // 2
                        sl = slice(offs[t] + lo, offs[t] + hi)
                        eng[q].dma_start(es[t][:, lo:hi], eD[:, sl])
                elif tok[0] == "Y":
                    t = int(tok[1:])
                    F = plan[t]
                    y_t = ypool.tile([P, F], f32, tag="y")
                    ye = nc.vector if y_eng[t] == "d" else nc.gpsimd
                    ye.tensor_tensor(y_t[:], preds[t][:], es[t][:], Alu.mult)
                    ycache[t] = y_t
                elif tok[0] == "T":
                    t = int(tok[1:])
                    F = plan[t]
                    jb = jpool.tile([P, F], bf16, tag="jb")
                    nc.vector.tensor_scalar(
                        jb[:], es[t][:], 0.0, 0.0, Alu.max, Alu.add,
                        accum_out=acc[:, t : t + 1])
                elif tok[0] == "A":
                    t = int(tok[1:])
                    F = plan[t]
                    z_t = zpool.tile([P, F], f32r, tag="z")
                    nc.scalar.activation(z_t[:], ycache[t][:], Act.Ln,
                                         bias=half[:, 0:1], scale=1.0)
                    zcache[t] = z_t
                elif tok.startswith("ZM"):
                    t = int(tok[2:])
                    F = plan[t]
                    cw = F // 4
                    if first_zm[0]:
                        assert cw == 512, "first z-matmul must be full width"
                    for c in range(4):
                        nc.tensor.matmul(
                            psz[0:1, 512 * c : 512 * c + cw],
                            onesf[:, 0:1].bitcast(f32r),
                            zcache[t][:, c * cw : (c + 1) * cw],
                            start=first_zm[0], stop=(ti == last_zm),
                            skip_group_check=True)
                    first_zm[0] = False
                elif tok[0] == "M":
                    t = int(tok[1:])
                    F = plan[t]
                    cw = F // 4
                    if first_m[0]:
                        assert cw == 512, "first e-matmul must be full width"
                    for c in range(4):
                        nc.tensor.matmul(
                            pse[0:1, 512 * c : 512 * c + cw],
                            ones[:, 0:1], es[t][:, c * cw : (c + 1) * cw],
                            start=first_m[0], stop=(ti == last_m),
                            skip_group_check=True)
                    first_m[0] = False
                elif tok == "FE" or tok == "FZ":
                    ps = pse if tok == "FE" else psz
                    base = nt if tok == "FE" else nt + 4
                    for c in range(4):
                        on_act = (tok == "FZ" and c >= 2)
                        jf = jpool.tile([1, 512], f32,
                                        tag="jfa" if on_act else "jfd")
                        if on_act:
                            nc.scalar.activation(
                                jf[0:1, :], ps[0:1, 512 * c : 512 * c + 512],
                                Act.Copy,
                                accum_out=acc[0:1, base + c : base + c + 1])
                        else:
                            nc.vector.tensor_scalar(
                                jf[0:1, :], ps[0:1, 512 * c : 512 * c + 512],
                                0.0, 0.0, Alu.add, Alu.add,
                                accum_out=acc[0:1, base + c : base + c + 1])
                elif tok == "STATS":
                    nc.sync.dma_start(outD[:], acc[:])
                else:
                    raise ValueError(tok)
    nc.finalize()
    _NC_CACHE[key] = nc
    return nc


def _final_scalar(sw, sn, zsum, pred=None, gt=None, mask=None):
    """Host-side merge of per-core sums into the balance loss (f64)."""
    n_ignored = float(TOT) - (sw + sn)
    total_loss = -(zsum - n_ignored * _ln_half_bf16())  # pos + all-neg loss
    neg_count = min(sn, NEG_RATIO * sw)
    if neg_count >= sn:
        num = total_loss
    else:
        # exact OHEM fallback (not triggered for the shipped distribution)
        p = np.asarray(pred, dtype=np.float64).ravel()
        g = np.asarray(gt, dtype=np.float64).ravel()
        m = np.asarray(mask, dtype=np.float64).ravel()
        pos_loss = -(g * m * np.log(p)).sum()
        neg_loss = (1.0 - g) * m * (-np.log1p(-p))
        k = int(neg_count)
        if k <= 0:
            topk = 0.0
        else:
            part = np.partition(neg_loss, neg_loss.size - k)
            topk = float(part[neg_loss.size - k:].sum())
        num = pos_loss + topk
    if neg_count > 0:
        out = num / (sw + neg_count + EPS)
    else:
        out = num / (sw + EPS)
    return np.asarray(out, dtype=np.float32).reshape(())


def _encode(pred, gt, mask):
    import ml_dtypes
    # centered probabilities: the device computes Ln((pred-1/2)*e + 1/2);
    # shifting on the host keeps the on-device y a single tensor_tensor mult.
    predf = np.ascontiguousarray(
        np.asarray(pred, dtype=np.float32) - np.float32(0.5)).reshape(
        N_CORES, P, W)
    e = (np.asarray(mask, dtype=np.float32)
         * (2.0 * np.asarray(gt, dtype=np.float32) - 1.0))
    e = np.ascontiguousarray(e.astype(ml_dtypes.bfloat16)).reshape(N_CORES, P, W)
    return predf, e


def run_device(pred, gt, mask, trace=False, **run_kwargs):
    _ensure_concourse()
    from concourse.bass_utils import run_bass_kernel_spmd

    nc = _build_nc()
    predf, e = _encode(pred, gt, mask)
    in_maps = [{"pred": predf[i], "e": e[i]} for i in range(N_CORES)]
    res = run_bass_kernel_spmd(nc, in_maps, list(range(N_CORES)), trace=trace,
                               **run_kwargs)
    stats = np.stack([np.asarray(r["stats"], dtype=np.float64)
                      for r in res.results])
    sw = stats[:, :, 0:NT].sum()
    se = stats[:, 0, NT:NT + 4].sum()
    zsum = stats[:, 0, NT + 4:NT + 8].sum()
    return (sw, sw - se, zsum), res


def kernel(pred, gt, mask):
    pred = np.asarray(pred, dtype=np.float32)
    gt = np.asarray(gt, dtype=np.float32)
    mask = np.asarray(mask, dtype=np.float32)
    if pred.shape != FULL_SHAPE:
        # defensive pure-host path for non-conforming shapes
        p64 = pred.astype(np.float64)
        sw = float((gt * mask).sum(dtype=np.float64))
        sn = float(((1.0 - gt) * mask).sum(dtype=np.float64))
        total = -(gt * mask * np.log(p64)
                  + (1.0 - gt) * mask * np.log1p(-p64)).sum()
        neg_count = min(sn, NEG_RATIO * sw)
        out = (total / (sw + neg_count + EPS) if neg_count > 0
               else total / (sw + EPS))
        return np.asarray(out, dtype=np.float32).reshape(())
    (se, sm, zsum), _ = run_device(pred, gt, mask)
    return _final_scalar(se, sm, zsum, pred, gt, mask)


# revision 3
# speedup vs baseline: 1.0496x; 1.0496x over previous
"""BalanceLoss (BCE + OHEM top-k negatives) on 8 trn2 NeuronCores — v2.

Algorithm
---------
Host encodes the two {0,1} label tensors as one categorical bf16 tensor
    e = mask * (2*gt - 1)  in {-1, 0, +1}   (pos / ignore / neg label)
Per core (data-parallel shard of 1/8 of the elements, [128 x 12800]):
    y = pred_centered * e                   (tensor_tensor, Pool/DVE;
                                             host ships pred - 1/2)
    z = Ln(y + 1/2)                         (ScalarE)
      = ln(pred)   where e=+1   (positive, masked-in)
      = ln(1-pred) where e=-1   (negative)
      = ln(1/2)    where e= 0   (masked-out; exact, host-corrected)
    z tiles are float32r so the PE column-sum runs at 1 cycle/row without
    bf16 rounding bias.
    Sz  = sum(z)       PE ones-matmul column sums into PSUM banks 0-3
    Se  = sum(e)       PE ones-matmul column sums into PSUM banks 4-7
    sw  = sum(relu(e)) DVE tensor_scalar(max 0) + accum (4x bf16 rate)
Host merge (exact f64): sn = sw - Se, Sm = sw + sn,
    pos_loss+neg_loss_all = -(Sz - (N-Sm)*ln(1/2)_bf16)
OHEM top-k == all-negatives whenever k = min(sn, 3*sw) == sn (true for this
distribution); exact host fallback otherwise.

Scheduling: all pred/e tiles stay resident in SBUF (no buffer recycling), and
the program is emitted from an explicit token sequence so each engine queue
(SP/Act/Pool DMA+compute, DVE, PE) receives jobs in a hand-tuned order that
keeps the ScalarE Ln stream — the longest serial chain — fed without stalls.
"""

import os
import sys

import numpy as np

# ---------------------------------------------------------------- constants
FULL_SHAPE = (32, 1, 640, 640)
TOT = 32 * 640 * 640          # 13_107_200 elements
N_CORES = 8
PER_CORE = TOT // N_CORES     # 1_638_400
P = 128                       # SBUF partitions
W = PER_CORE // P             # 12_800 free-dim elements per partition
NEG_RATIO = 3.0
EPS = 1e-6

TILE_PLAN = (128, 1024, 1408, 1792, 2048, 2048, 2048, 2048, 256)
assert sum(TILE_PLAN) == W
NT = len(TILE_PLAN)

# y-engine per tile: 'd' = DVE, 'g' = Pool
Y_ENG = ("g", "g", "d", "d", "d", "d", "d", "g", "g")

# Emission order. Tokens:
#   P<t>@<q>   pred-tile DMA on queue q (s/a/g); Pa/Pb = first/second half
#   E<t>@<q>   e-tile DMA
#   Y<t>       y stt (engine from Y_ENG)
#   T<t>       sum(relu(e)) = pos_count tensor_scalar on DVE
#   M<t>       PE e-matmuls for tile t
#   A<t>       Ln act on ScalarE
#   ZM<t>      PE z-matmuls for tile t (needs A<t> first)
#   FE / FZ    psum folds (e / z streams; split over Pool+DVE)
#   STATS      final stats DMA on SP
# PE stream init: M3/ZM3 are emitted first on PE with start=True (tile 3 is
# full-width, so its write initializes all 512 cols of each psum group).
SCHEDULE = (
    "P0@s", "E0@a", "P1@g", "E1@a", "Pa2@s", "Pb2@a", "E2@g",
    "Y0", "T0", "A0",
    "Pa3@s", "Pb3@g", "E3@g",
    "Y1", "T1", "A1",
    "Pa4@s", "Pb4@g", "E4@s", "M4", "M0", "M1",
    "Y2", "T2", "A2",
    "Pa5@s", "Pb5@g", "E5@s", "M2", "M3",
    "Y3", "T3", "A3",
    "Pa6@s", "Pb6@g", "E6@s",
    "Y4", "T4", "A4", "ZM4", "ZM0", "ZM1", "ZM2", "ZM3",
    "Pa7@s", "Pb7@g", "Ea7@s", "Eb7@g", "M5",
    "Y5", "T5", "A5", "ZM5",
    "P8@s", "E8@s", "M6",
    "Y6", "T6", "A6", "ZM6",
    "M7", "M8",
    "Y7", "T7", "A7", "ZM7",
    "Y8", "T8", "A8", "ZM8",
    "FE", "FZ", "STATS",
)

_CONCOURSE_PATHS = ("/opt/trn_rl_repo", "/root/.axon_site/_ro/trn_rl_repo")


def _ensure_concourse():
    try:
        import concourse.bass  # noqa: F401
    except ImportError:
        for p in _CONCOURSE_PATHS:
            if os.path.isdir(p) and p not in sys.path:
                sys.path.insert(0, p)
        import concourse.bass  # noqa: F401


_NC_CACHE = {}

# ln(0.5) in fp32 — matches the device z value for masked-out elements
# (the Ln argument is exactly 0.5 there; z tiles are fp32(r)).
def _ln_half_bf16():
    return float(np.float32(np.log(np.float32(0.5))))


def _build_nc(plan=TILE_PLAN, y_eng=Y_ENG, schedule=SCHEDULE):
    key = (plan, y_eng, schedule)
    if key in _NC_CACHE:
        return _NC_CACHE[key]
    _ensure_concourse()
    import concourse.bacc as bacc
    import concourse.mybir as mybir
    import concourse.tile as tile

    f32 = mybir.dt.float32
    f32r = mybir.dt.float32r
    bf16 = mybir.dt.bfloat16
    Act = mybir.ActivationFunctionType
    Alu = mybir.AluOpType

    nt = len(plan)
    offs = [sum(plan[:i]) for i in range(nt)]
    last_m = max(i for i, tok in enumerate(schedule) if tok.startswith("M"))
    last_zm = max(i for i, tok in enumerate(schedule) if tok.startswith("ZM"))
    # stats: [0:nt]=sum(|e|) per tile, [nt:nt+4]=Se folds, [nt+4:nt+8]=Sz folds
    SCOLS = nt + 8

    nc = bacc.Bacc(None, target_bir_lowering=False)
    predD = nc.declare_dram_parameter("pred", [P, W], f32, isOutput=False)
    eD = nc.declare_dram_parameter("e", [P, W], bf16, isOutput=False)
    outD = nc.declare_dram_parameter("stats", [P, SCOLS], f32, isOutput=True)

    with tile.TileContext(nc) as tc:
        eng = {"s": nc.sync, "a": nc.scalar, "g": nc.gpsimd}
        with (
            tc.tile_pool(name="iop", bufs=1) as iop,
            tc.tile_pool(name="ioe", bufs=1) as ioe,
            tc.tile_pool(name="ytmp", bufs=3) as ypool,
            tc.tile_pool(name="ztmp", bufs=6) as zpool,
            tc.tile_pool(name="junk", bufs=2) as jpool,
            tc.tile_pool(name="accp", bufs=1) as acc_pool,
            tc.tile_pool(name="ps", bufs=1, space="PSUM") as ps_pool,
        ):
            acc = acc_pool.tile([P, SCOLS], f32)
            nc.vector.memset(acc[:], 0.0)
            ones = acc_pool.tile([P, 1], bf16)
            nc.gpsimd.memset(ones[:], 1.0)
            onesf = acc_pool.tile([P, 1], f32)
            nc.gpsimd.memset(onesf[:], 1.0)
            half = acc_pool.tile([P, 1], f32)
            nc.gpsimd.memset(half[:], 0.5)
            # 8 psum banks: z-stream groups 0-3 (cols 512c), e-stream 4-7.
            # The first-emitted matmul of each stream is a full-width tile
            # with start=True, initializing all 512 cols of each group.
            psz = ps_pool.tile([1, 2048], f32, tag="psz")
            pse = ps_pool.tile([1, 2048], f32, tag="pse")

            preds = [iop.tile([P, F], f32, tag=f"pred{t}", name=f"pred{t}")
                     for t, F in enumerate(plan)]
            es = [ioe.tile([P, F], bf16, tag=f"e{t}", name=f"e{t}")
                  for t, F in enumerate(plan)]

            first_m = [True]
            first_zm = [True]
            ycache = {}
            zcache = {}

            for ti, tok in enumerate(schedule):
                if "@" in tok:
                    q = tok[-1]
                    body = tok[: tok.index("@")]
                    if body[0] == "P":
                        half_sel = None
                        if body[1] in "ab":
                            half_sel, t = body[1], int(body[2:])
                        else:
                            t = int(body[1:])
                        F = plan[t]
                        lo, hi = 0, F
                        if half_sel == "a":
                            hi = F // 2
                        elif half_sel == "b":
                            lo = F // 2
                        sl = slice(offs[t] + lo, offs[t] + hi)
                        eng[q].dma_start(preds[t][:, lo:hi], predD[:, sl])
                    else:
                        half_sel = None
                        if body[1] in "ab":
                            half_sel, t = body[1], int(body[2:])
                        else:
                            t = int(body[1:])
                        F = plan[t]
                        lo, hi = 0, F
                        if half_sel == "a":
                            hi = F // 2
                        elif half_sel == "b":
                            lo = F // 2
                        sl = slice(offs[t] + lo, offs[t] + hi)
                        eng[q].dma_start(es[t][:, lo:hi], eD[:, sl])
                elif tok[0] == "Y":
                    t = int(tok[1:])
                    F = plan[t]
                    y_t = ypool.tile([P, F], f32, tag="y")
                    ye = nc.vector if y_eng[t] == "d" else nc.gpsimd
                    ye.tensor_tensor(y_t[:], preds[t][:], es[t][:], Alu.mult)
                    ycache[t] = y_t
                elif tok[0] == "T":
                    t = int(tok[1:])
                    F = plan[t]
                    jb = jpool.tile([P, F], bf16, tag="jb")
                    nc.vector.tensor_scalar(
                        jb[:], es[t][:], 0.0, 0.0, Alu.max, Alu.add,
                        accum_out=acc[:, t : t + 1])
                elif tok[0] == "A":
                    t = int(tok[1:])
                    F = plan[t]
                    z_t = zpool.tile([P, F], f32r, tag="z")
                    nc.scalar.activation(z_t[:], ycache[t][:], Act.Ln,
                                         bias=half[:, 0:1], scale=1.0)
                    zcache[t] = z_t
                elif tok.startswith("ZM"):
                    t = int(tok[2:])
                    F = plan[t]
                    cw = F // 4
                    if first_zm[0]:
                        assert cw == 512, "first z-matmul must be full width"
                    for c in range(4):
                        nc.tensor.matmul(
                            psz[0:1, 512 * c : 512 * c + cw],
                            onesf[:, 0:1].bitcast(f32r),
                            zcache[t][:, c * cw : (c + 1) * cw],
                            start=first_zm[0], stop=(ti == last_zm),
                            skip_group_check=True)
                    first_zm[0] = False
                elif tok[0] == "M":
                    t = int(tok[1:])
                    F = plan[t]
                    cw = F // 4
                    if first_m[0]:
                        assert cw == 512, "first e-matmul must be full width"
                    for c in range(4):
                        nc.tensor.matmul(
                            pse[0:1, 512 * c : 512 * c + cw],
                            ones[:, 0:1], es[t][:, c * cw : (c + 1) * cw],
                            start=first_m[0], stop=(ti == last_m),
                            skip_group_check=True)
                    first_m[0] = False
                elif tok == "FE" or tok == "FZ":
                    ps = pse if tok == "FE" else psz
                    base = nt if tok == "FE" else nt + 4
                    for c in range(4):
                        on_act = (tok == "FZ" and c >= 2)
                        jf = jpool.tile([1, 512], f32,
                                        tag="jfa" if on_act else "jfd")
                        if on_act:
                            nc.scalar.activation(
                                jf[0:1, :], ps[0:1, 512 * c : 512 * c + 512],
                                Act.Copy,
                                accum_out=acc[0:1, base + c : base + c + 1])
                        else:
                            nc.vector.tensor_scalar(
                                jf[0:1, :], ps[0:1, 512 * c : 512 * c + 512],
                                0.0, 0.0, Alu.add, Alu.add,
                                accum_out=acc[0:1, base + c : base + c + 1])
                elif tok == "STATS":
                    nc.sync.dma_start(outD[:], acc[:])
                else:
                    raise ValueError(tok)
    nc.finalize()
    _NC_CACHE[key] = nc
    return nc


def _final_scalar(sw, sn, zsum, pred=None, gt=None, mask=None):
    """Host-side merge of per-core sums into the balance loss (f64)."""
    n_ignored = float(TOT) - (sw + sn)
    total_loss = -(zsum - n_ignored * _ln_half_bf16())  # pos + all-neg loss
    neg_count = min(sn, NEG_RATIO * sw)
    if neg_count >= sn:
        num = total_loss
    else:
        # exact OHEM fallback (not triggered for the shipped distribution)
        p = np.asarray(pred, dtype=np.float64).ravel()
        g = np.asarray(gt, dtype=np.float64).ravel()
        m = np.asarray(mask, dtype=np.float64).ravel()
        pos_loss = -(g * m * np.log(p)).sum()
        neg_loss = (1.0 - g) * m * (-np.log1p(-p))
        k = int(neg_count)
        if k <= 0:
            topk = 0.0
        else:
            part = np.partition(neg_loss, neg_loss.size - k)
            topk = float(part[neg_loss.size - k:].sum())
        num = pos_loss + topk
    if neg_count > 0:
        out = num / (sw + neg_count + EPS)
    else:
        out = num / (sw + EPS)
    return np.asarray(out, dtype=np.float32).reshape(())


def _encode(pred, gt, mask):
    import ml_dtypes
    # centered probabilities: the device computes Ln((pred-1/2)*e + 1/2);
    # shifting on the host keeps the on-device y a single tensor_tensor mult.
    predf = np.ascontiguousarray(
        np.asarray(pred, dtype=np.float32) - np.float32(0.5)).reshape(
        N_CORES, P, W)
    e = (np.asarray(mask, dtype=np.float32)
         * (2.0 * np.asarray(gt, dtype=np.float32) - 1.0))
    e = np.ascontiguousarray(e.astype(ml_dtypes.bfloat16)).reshape(N_CORES, P, W)
    return predf, e


def run_device(pred, gt, mask, trace=False, **run_kwargs):
    _ensure_concourse()
    from concourse.bass_utils import run_bass_kernel_spmd

    nc = _build_nc()
    predf, e = _encode(pred, gt, mask)
    in_maps = [{"pred": predf[i], "e": e[i]} for i in range(N_CORES)]
    res = run_bass_kernel_spmd(nc, in_maps, list(range(N_CORES)), trace=trace,
                               **run_kwargs)
    stats = np.stack([np.asarray(r["stats"], dtype=np.float64)
                      for r in res.results])
    sw = stats[:, :, 0:NT].sum()
    se = stats[:, 0, NT:NT + 4].sum()
    zsum = stats[:, 0, NT + 4:NT + 8].sum()
    return (sw, sw - se, zsum), res


def kernel(pred, gt, mask):
    pred = np.asarray(pred, dtype=np.float32)
    gt = np.asarray(gt, dtype=np.float32)
    mask = np.asarray(mask, dtype=np.float32)
    if pred.shape != FULL_SHAPE:
        # defensive pure-host path for non-conforming shapes
        p64 = pred.astype(np.float64)
        sw = float((gt * mask).sum(dtype=np.float64))
        sn = float(((1.0 - gt) * mask).sum(dtype=np.float64))
        total = -(gt * mask * np.log(p64)
                  + (1.0 - gt) * mask * np.log1p(-p64)).sum()
        neg_count = min(sn, NEG_RATIO * sw)
        out = (total / (sw + neg_count + EPS) if neg_count > 0
               else total / (sw + EPS))
        return np.asarray(out, dtype=np.float32).reshape(())
    (se, sm, zsum), _ = run_device(pred, gt, mask)
    return _final_scalar(se, sm, zsum, pred, gt, mask)


# revision 4
# speedup vs baseline: 1.1234x; 1.0703x over previous
"""BalanceLoss (BCE + OHEM top-k negatives) on 8 trn2 NeuronCores — v2.

Algorithm
---------
Host encodes the two {0,1} label tensors as one categorical bf16 tensor
    e = mask * (2*gt - 1)  in {-1, 0, +1}   (pos / ignore / neg label)
Per core (data-parallel shard of 1/8 of the elements, [128 x 12800]):
    y = pred_centered * e                   (tensor_tensor, Pool/DVE;
                                             host ships pred - 1/2)
    z = Ln(y + 1/2)                         (ScalarE)
      = ln(pred)   where e=+1   (positive, masked-in)
      = ln(1-pred) where e=-1   (negative)
      = ln(1/2)    where e= 0   (masked-out; exact, host-corrected)
    z tiles are float32r so the PE column-sum runs at 1 cycle/row without
    bf16 rounding bias.
    Sz  = sum(z)       PE ones-matmul column sums into PSUM banks 0-3
    Se  = sum(e)       PE ones-matmul column sums into PSUM banks 4-7
    sw  = sum(relu(e)) DVE tensor_scalar(max 0) + accum (4x bf16 rate)
Host merge (exact f64): sn = sw - Se, Sm = sw + sn,
    pos_loss+neg_loss_all = -(Sz - (N-Sm)*ln(1/2)_bf16)
OHEM top-k == all-negatives whenever k = min(sn, 3*sw) == sn (true for this
distribution); exact host fallback otherwise.

Scheduling: all pred/e tiles stay resident in SBUF (no buffer recycling), and
the program is emitted from an explicit token sequence so each engine queue
(SP/Act/Pool DMA+compute, DVE, PE) receives jobs in a hand-tuned order that
keeps the ScalarE Ln stream — the longest serial chain — fed without stalls.
"""

import os
import sys

import numpy as np

# ---------------------------------------------------------------- constants
FULL_SHAPE = (32, 1, 640, 640)
TOT = 32 * 640 * 640          # 13_107_200 elements
N_CORES = 8
PER_CORE = TOT // N_CORES     # 1_638_400
P = 128                       # SBUF partitions
W = PER_CORE // P             # 12_800 free-dim elements per partition
NEG_RATIO = 3.0
EPS = 1e-6

TILE_PLAN = (128, 1024, 1408, 1792, 2048, 2048, 2048, 2048, 256)
assert sum(TILE_PLAN) == W
NT = len(TILE_PLAN)

# y-engine per tile: 'd' = DVE, 'g' = Pool
Y_ENG = ("g", "g", "d", "d", "d", "d", "d", "g", "g")

# Emission order. Tokens:
#   P<t>@<q>   pred-tile DMA on queue q (s/a/g); Pa/Pb = first/second half
#   E<t>@<q>   e-tile DMA
#   Y<t>       y stt (engine from Y_ENG)
#   T<t>       sum(relu(e)) = pos_count tensor_scalar on DVE
#   M<t>       PE e-matmuls for tile t
#   A<t>       Ln act on ScalarE
#   ZM<t>      PE z-matmuls for tile t (needs A<t> first)
#   FE / FZ    psum folds (e / z streams; split over Pool+DVE)
#   STATS      final stats DMA on SP
# PE stream init: M4/ZM4 are emitted first on PE with start=True (tile 4 is
# full-width 2048, so its write initializes all 512 cols of each psum group).
SCHEDULE = (
    "P0@s", "E0@a", "P1@g", "E1@a", "Pa2@s", "Pb2@a", "E2@g",
    "Y0", "T0", "A0",
    "Pa3@s", "Pb3@g", "E3@g",
    "Y1", "T1", "A1",
    "Pa4@s", "Pb4@g", "E4@s", "M4", "M0", "M1",
    "Y2", "T2", "A2",
    "Pa5@s", "Pb5@g", "E5@s", "M2", "M3",
    "Y3", "T3", "A3",
    "Pa6@s", "Pb6@g", "E6@s",
    "Y4", "T4", "A4", "ZM4", "ZM0", "ZM1", "ZM2", "ZM3",
    "Pa7@s", "Pb7@g", "Ea7@s", "Eb7@g", "M5",
    "Y5", "T5", "A5", "ZM5",
    "P8@s", "E8@s", "M6",
    "Y6", "T6", "A6", "ZM6",
    "M7", "M8",
    "Y7", "T7", "A7", "ZM7",
    "Y8", "T8", "A8", "ZM8",
    "FE", "FZ", "STATS",
)

_CONCOURSE_PATHS = ("/opt/trn_rl_repo", "/root/.axon_site/_ro/trn_rl_repo")


def _ensure_concourse():
    try:
        import concourse.bass  # noqa: F401
    except ImportError:
        for p in _CONCOURSE_PATHS:
            if os.path.isdir(p) and p not in sys.path:
                sys.path.insert(0, p)
        import concourse.bass  # noqa: F401


_NC_CACHE = {}

# ln(0.5) in fp32 — matches the device z value for masked-out elements
# (the Ln argument is exactly 0.5 there; z tiles are fp32(r)).
def _ln_half_bf16():
    return float(np.float32(np.log(np.float32(0.5))))


def _build_nc(plan=TILE_PLAN, y_eng=Y_ENG, schedule=SCHEDULE):
    key = (plan, y_eng, schedule)
    if key in _NC_CACHE:
        return _NC_CACHE[key]
    _ensure_concourse()
    import concourse.bacc as bacc
    import concourse.mybir as mybir
    import concourse.tile as tile

    f32 = mybir.dt.float32
    f32r = mybir.dt.float32r
    bf16 = mybir.dt.bfloat16
    Act = mybir.ActivationFunctionType
    Alu = mybir.AluOpType

    nt = len(plan)
    offs = [sum(plan[:i]) for i in range(nt)]
    last_m = max(i for i, tok in enumerate(schedule) if tok.startswith("M"))
    last_zm = max(i for i, tok in enumerate(schedule) if tok.startswith("ZM"))
    # stats: [0:nt]=sum(|e|) per tile, [nt:nt+4]=Se folds, [nt+4:nt+8]=Sz folds
    SCOLS = nt + 8

    nc = bacc.Bacc(None, target_bir_lowering=False)
    predD = nc.declare_dram_parameter("pred", [P, W], f32, isOutput=False)
    eD = nc.declare_dram_parameter("e", [P, W], bf16, isOutput=False)
    outD = nc.declare_dram_parameter("stats", [P, SCOLS], f32, isOutput=True)

    with tile.TileContext(nc) as tc:
        eng = {"s": nc.sync, "a": nc.scalar, "g": nc.gpsimd}
        with (
            tc.tile_pool(name="iop", bufs=1) as iop,
            tc.tile_pool(name="ioe", bufs=1) as ioe,
            tc.tile_pool(name="ytmp", bufs=3) as ypool,
            tc.tile_pool(name="ztmp", bufs=6) as zpool,
            tc.tile_pool(name="junk", bufs=2) as jpool,
            tc.tile_pool(name="accp", bufs=1) as acc_pool,
            tc.tile_pool(name="ps", bufs=1, space="PSUM") as ps_pool,
        ):
            acc = acc_pool.tile([P, SCOLS], f32)
            nc.vector.memset(acc[:], 0.0)
            ones = acc_pool.tile([P, 1], bf16)
            nc.gpsimd.memset(ones[:], 1.0)
            onesf = acc_pool.tile([P, 1], f32)
            nc.gpsimd.memset(onesf[:], 1.0)
            half = acc_pool.tile([P, 1], f32)
            nc.gpsimd.memset(half[:], 0.5)
            # 8 psum banks: z-stream groups 0-3 (cols 512c), e-stream 4-7.
            # The first-emitted matmul of each stream is a full-width tile
            # with start=True, initializing all 512 cols of each group.
            psz = ps_pool.tile([1, 2048], f32, tag="psz")
            pse = ps_pool.tile([1, 2048], f32, tag="pse")

            preds = [iop.tile([P, F], f32, tag=f"pred{t}", name=f"pred{t}")
                     for t, F in enumerate(plan)]
            es = [ioe.tile([P, F], bf16, tag=f"e{t}", name=f"e{t}")
                  for t, F in enumerate(plan)]

            first_m = [True]
            first_zm = [True]
            ycache = {}
            zcache = {}

            for ti, tok in enumerate(schedule):
                if "@" in tok:
                    q = tok[-1]
                    body = tok[: tok.index("@")]
                    if body[0] == "P":
                        half_sel = None
                        if body[1] in "ab":
                            half_sel, t = body[1], int(body[2:])
                        else:
                            t = int(body[1:])
                        F = plan[t]
                        lo, hi = 0, F
                        if half_sel == "a":
                            hi = F // 2
                        elif half_sel == "b":
                            lo = F // 2
                        sl = slice(offs[t] + lo, offs[t] + hi)
                        eng[q].dma_start(preds[t][:, lo:hi], predD[:, sl])
                    else:
                        half_sel = None
                        if body[1] in "ab":
                            half_sel, t = body[1], int(body[2:])
                        else:
                            t = int(body[1:])
                        F = plan[t]
                        lo, hi = 0, F
                        if half_sel == "a":
                            hi = F // 2
                        elif half_sel == "b":
                            lo = F // 2
                        sl = slice(offs[t] + lo, offs[t] + hi)
                        eng[q].dma_start(es[t][:, lo:hi], eD[:, sl])
                elif tok[0] == "Y":
                    t = int(tok[1:])
                    F = plan[t]
                    y_t = ypool.tile([P, F], f32, tag="y")
                    ye = nc.vector if y_eng[t] == "d" else nc.gpsimd
                    ye.tensor_tensor(y_t[:], preds[t][:], es[t][:], Alu.mult)
                    ycache[t] = y_t
                elif tok[0] == "T":
                    t = int(tok[1:])
                    F = plan[t]
                    jb = jpool.tile([P, F], bf16, tag="jb")
                    nc.vector.tensor_scalar(
                        jb[:], es[t][:], 0.0, 0.0, Alu.max, Alu.add,
                        accum_out=acc[:, t : t + 1])
                elif tok[0] == "A":
                    t = int(tok[1:])
                    F = plan[t]
                    z_t = zpool.tile([P, F], f32r, tag="z")
                    nc.scalar.activation(z_t[:], ycache[t][:], Act.Ln,
                                         bias=half[:, 0:1], scale=1.0)
                    zcache[t] = z_t
                elif tok.startswith("ZM"):
                    t = int(tok[2:])
                    F = plan[t]
                    cw = F // 4
                    if first_zm[0]:
                        assert cw == 512, "first z-matmul must be full width"
                    for c in range(4):
                        nc.tensor.matmul(
                            psz[0:1, 512 * c : 512 * c + cw],
                            onesf[:, 0:1].bitcast(f32r),
                            zcache[t][:, c * cw : (c + 1) * cw],
                            start=first_zm[0], stop=(ti == last_zm),
                            skip_group_check=True)
                    first_zm[0] = False
                elif tok[0] == "M":
                    t = int(tok[1:])
                    F = plan[t]
                    cw = F // 4
                    if first_m[0]:
                        assert cw == 512, "first e-matmul must be full width"
                    for c in range(4):
                        nc.tensor.matmul(
                            pse[0:1, 512 * c : 512 * c + cw],
                            ones[:, 0:1], es[t][:, c * cw : (c + 1) * cw],
                            start=first_m[0], stop=(ti == last_m),
                            skip_group_check=True)
                    first_m[0] = False
                elif tok == "FE" or tok == "FZ":
                    ps = pse if tok == "FE" else psz
                    base = nt if tok == "FE" else nt + 4
                    for c in range(4):
                        on_act = (tok == "FZ" and c >= 2)
                        jf = jpool.tile([1, 512], f32,
                                        tag="jfa" if on_act else "jfd")
                        if on_act:
                            nc.scalar.activation(
                                jf[0:1, :], ps[0:1, 512 * c : 512 * c + 512],
                                Act.Copy,
                                accum_out=acc[0:1, base + c : base + c + 1])
                        else:
                            nc.vector.tensor_scalar(
                                jf[0:1, :], ps[0:1, 512 * c : 512 * c + 512],
                                0.0, 0.0, Alu.add, Alu.add,
                                accum_out=acc[0:1, base + c : base + c + 1])
                elif tok == "STATS":
                    nc.sync.dma_start(outD[:], acc[:])
                else:
                    raise ValueError(tok)
    nc.finalize()
    _NC_CACHE[key] = nc
    return nc


def _final_scalar(sw, sn, zsum, pred=None, gt=None, mask=None):
    """Host-side merge of per-core sums into the balance loss (f64)."""
    n_ignored = float(TOT) - (sw + sn)
    total_loss = -(zsum - n_ignored * _ln_half_bf16())  # pos + all-neg loss
    neg_count = min(sn, NEG_RATIO * sw)
    if neg_count >= sn:
        num = total_loss
    else:
        # exact OHEM fallback (not triggered for the shipped distribution)
        p = np.asarray(pred, dtype=np.float64).ravel()
        g = np.asarray(gt, dtype=np.float64).ravel()
        m = np.asarray(mask, dtype=np.float64).ravel()
        pos_loss = -(g * m * np.log(p)).sum()
        neg_loss = (1.0 - g) * m * (-np.log1p(-p))
        k = int(neg_count)
        if k <= 0:
            topk = 0.0
        else:
            part = np.partition(neg_loss, neg_loss.size - k)
            topk = float(part[neg_loss.size - k:].sum())
        num = pos_loss + topk
    if neg_count > 0:
        out = num / (sw + neg_count + EPS)
    else:
        out = num / (sw + EPS)
    return np.asarray(out, dtype=np.float32).reshape(())


def _encode(pred, gt, mask):
    import ml_dtypes
    # centered probabilities: the device computes Ln((pred-1/2)*e + 1/2);
    # shifting on the host keeps the on-device y a single tensor_tensor mult.
    predf = np.ascontiguousarray(
        np.asarray(pred, dtype=np.float32) - np.float32(0.5)).reshape(
        N_CORES, P, W)
    e = (np.asarray(mask, dtype=np.float32)
         * (2.0 * np.asarray(gt, dtype=np.float32) - 1.0))
    e = np.ascontiguousarray(e.astype(ml_dtypes.bfloat16)).reshape(N_CORES, P, W)
    return predf, e


def run_device(pred, gt, mask, trace=False, **run_kwargs):
    _ensure_concourse()
    from concourse.bass_utils import run_bass_kernel_spmd

    nc = _build_nc()
    predf, e = _encode(pred, gt, mask)
    in_maps = [{"pred": predf[i], "e": e[i]} for i in range(N_CORES)]
    res = run_bass_kernel_spmd(nc, in_maps, list(range(N_CORES)), trace=trace,
                               **run_kwargs)
    stats = np.stack([np.asarray(r["stats"], dtype=np.float64)
                      for r in res.results])
    sw = stats[:, :, 0:NT].sum()
    se = stats[:, 0, NT:NT + 4].sum()
    zsum = stats[:, 0, NT + 4:NT + 8].sum()
    return (sw, sw - se, zsum), res


def kernel(pred, gt, mask):
    pred = np.asarray(pred, dtype=np.float32)
    gt = np.asarray(gt, dtype=np.float32)
    mask = np.asarray(mask, dtype=np.float32)
    if pred.shape != FULL_SHAPE:
        # defensive pure-host path for non-conforming shapes
        p64 = pred.astype(np.float64)
        sw = float((gt * mask).sum(dtype=np.float64))
        sn = float(((1.0 - gt) * mask).sum(dtype=np.float64))
        total = -(gt * mask * np.log(p64)
                  + (1.0 - gt) * mask * np.log1p(-p64)).sum()
        neg_count = min(sn, NEG_RATIO * sw)
        out = (total / (sw + neg_count + EPS) if neg_count > 0
               else total / (sw + EPS))
        return np.asarray(out, dtype=np.float32).reshape(())
    (se, sm, zsum), _ = run_device(pred, gt, mask)
    return _final_scalar(se, sm, zsum, pred, gt, mask)


# revision 5
# speedup vs baseline: 1.1579x; 1.0308x over previous
"""BalanceLoss (BCE + OHEM top-k negatives) on 8 trn2 NeuronCores — v2.

Algorithm
---------
Host encodes the two {0,1} label tensors as one categorical bf16 tensor
    e = mask * (2*gt - 1)  in {-1, 0, +1}   (pos / ignore / neg label)
Per core (data-parallel shard of 1/8 of the elements, [128 x 12800]):
    y = pred_centered * e                   (tensor_tensor, Pool/DVE;
                                             host ships pred - 1/2)
    z = Ln(y + 1/2)                         (ScalarE)
      = ln(pred)   where e=+1   (positive, masked-in)
      = ln(1-pred) where e=-1   (negative)
      = ln(1/2)    where e= 0   (masked-out; exact, host-corrected)
    z tiles are float32r so the PE column-sum runs at 1 cycle/row without
    bf16 rounding bias.
    Sz  = sum(z)       PE ones-matmul column sums into PSUM banks 0-3
    Se  = sum(e)       PE ones-matmul column sums into PSUM banks 4-7
    sw  = sum(relu(e)) DVE tensor_scalar(max 0) + accum (4x bf16 rate)
Host merge (exact f64): sn = sw - Se, Sm = sw + sn,
    pos_loss+neg_loss_all = -(Sz - (N-Sm)*ln(1/2)_bf16)
OHEM top-k == all-negatives whenever k = min(sn, 3*sw) == sn (true for this
distribution); exact host fallback otherwise.

Scheduling: all pred/e tiles stay resident in SBUF (no buffer recycling), and
the program is emitted from an explicit token sequence so each engine queue
(SP/Act/Pool DMA+compute, DVE, PE) receives jobs in a hand-tuned order that
keeps the ScalarE Ln stream — the longest serial chain — fed without stalls.
"""

import os
import sys

import numpy as np

# ---------------------------------------------------------------- constants
FULL_SHAPE = (32, 1, 640, 640)
TOT = 32 * 640 * 640          # 13_107_200 elements
N_CORES = 8
PER_CORE = TOT // N_CORES     # 1_638_400
P = 128                       # SBUF partitions
W = PER_CORE // P             # 12_800 free-dim elements per partition
NEG_RATIO = 3.0
EPS = 1e-6

TILE_PLAN = (128, 1024, 1408, 1792, 2048, 2048, 2048, 2048, 256)
assert sum(TILE_PLAN) == W
NT = len(TILE_PLAN)

# y-engine per tile: 'd' = DVE, 'g' = Pool
Y_ENG = ("d", "d", "d", "d", "g", "d", "d", "d", "d")

# Emission order. Tokens:
#   P<t>@<q>   pred-tile DMA on queue q (s/a/g); Pa/Pb = first/second half
#   E<t>@<q>   e-tile DMA
#   Y<t>       y stt (engine from Y_ENG)
#   T<t>       sum(relu(e)) = pos_count tensor_scalar on DVE
#   M<t>       PE e-matmuls for tile t
#   A<t>       Ln act on ScalarE
#   ZM<t>      PE z-matmuls for tile t (needs A<t> first)
#   FE / FZ    psum folds (e / z streams; split over Pool+DVE)
#   STATS      final stats DMA on SP
# PE stream init: M4/ZM4 are emitted first on PE with start=True (tile 4 is
# full-width 2048, so its write initializes all 512 cols of each psum group).
SCHEDULE = (
    "P0@s", "E0@a", "P1@g", "E1@a", "E2@s", "P2@s", "E3@g",
    "Y0", "T0", "A0",
    "P3@g",
    "Y1", "T1", "A1",
    "P4@s", "E4@g",
    "Y2", "T2", "A2",
    "P5@g", "E5@s", "M4", "M0", "M1",
    "Y3", "T3", "A3",
    "P6@s", "E6@g", "M2", "M3",
    "Y4", "T4", "A4", "ZM4", "ZM0", "ZM1", "ZM2", "ZM3",
    "P7@g", "E7@s", "M5",
    "P8@g", "E8@g", "M6", "M7", "M8",
    "Y5", "T5", "A5", "ZM5",
    "Y6", "T6", "A6", "ZM6",
    "Y8", "T8", "A8", "ZM8",
    "Y7", "T7", "A7", "ZM7",
    "FE", "FZ", "STATS",
)

_CONCOURSE_PATHS = ("/opt/trn_rl_repo", "/root/.axon_site/_ro/trn_rl_repo")


def _ensure_concourse():
    try:
        import concourse.bass  # noqa: F401
    except ImportError:
        for p in _CONCOURSE_PATHS:
            if os.path.isdir(p) and p not in sys.path:
                sys.path.insert(0, p)
        import concourse.bass  # noqa: F401


_NC_CACHE = {}

# ln(0.5) in fp32 — matches the device z value for masked-out elements
# (the Ln argument is exactly 0.5 there; z tiles are fp32(r)).
def _ln_half_bf16():
    return float(np.float32(np.log(np.float32(0.5))))


def _build_nc(plan=TILE_PLAN, y_eng=Y_ENG, schedule=SCHEDULE):
    key = (plan, y_eng, schedule)
    if key in _NC_CACHE:
        return _NC_CACHE[key]
    _ensure_concourse()
    import concourse.bacc as bacc
    import concourse.mybir as mybir
    import concourse.tile as tile

    f32 = mybir.dt.float32
    f32r = mybir.dt.float32r
    bf16 = mybir.dt.bfloat16
    f16 = mybir.dt.float16
    Act = mybir.ActivationFunctionType
    Alu = mybir.AluOpType

    nt = len(plan)
    offs = [sum(plan[:i]) for i in range(nt)]
    last_m = max(i for i, tok in enumerate(schedule) if tok.startswith("M"))
    last_zm = max(i for i, tok in enumerate(schedule) if tok.startswith("ZM"))
    # stats: [0:nt]=sum(|e|) per tile, [nt:nt+4]=Se folds, [nt+4:nt+8]=Sz folds
    SCOLS = nt + 8

    nc = bacc.Bacc(None, target_bir_lowering=False)
    predD = nc.declare_dram_parameter("pred", [P, W], f16, isOutput=False)
    eD = nc.declare_dram_parameter("e", [P, W], f16, isOutput=False)
    outD = nc.declare_dram_parameter("stats", [P, SCOLS], f32, isOutput=True)

    with tile.TileContext(nc) as tc:
        eng = {"s": nc.sync, "a": nc.scalar, "g": nc.gpsimd}
        with (
            tc.tile_pool(name="iop", bufs=1) as iop,
            tc.tile_pool(name="ioe", bufs=1) as ioe,
            tc.tile_pool(name="ytmp", bufs=6) as ypool,
            tc.tile_pool(name="ztmp", bufs=9) as zpool,
            tc.tile_pool(name="junk", bufs=2) as jpool,
            tc.tile_pool(name="accp", bufs=1) as acc_pool,
            tc.tile_pool(name="ps", bufs=1, space="PSUM") as ps_pool,
        ):
            acc = acc_pool.tile([P, SCOLS], f32)
            nc.vector.memset(acc[:], 0.0)
            ones = acc_pool.tile([P, 1], f16)
            nc.gpsimd.memset(ones[:], 1.0)
            onesf = acc_pool.tile([P, 1], f32)
            nc.gpsimd.memset(onesf[:], 1.0)
            half = acc_pool.tile([P, 1], f32)
            nc.gpsimd.memset(half[:], 0.5)
            # 8 psum banks: z-stream groups 0-3 (cols 512c), e-stream 4-7.
            # The first-emitted matmul of each stream is a full-width tile
            # with start=True, initializing all 512 cols of each group.
            psz = ps_pool.tile([1, 2048], f32, tag="psz")
            pse = ps_pool.tile([1, 2048], f32, tag="pse")

            preds = [iop.tile([P, F], f16, tag=f"pred{t}", name=f"pred{t}")
                     for t, F in enumerate(plan)]
            es = [ioe.tile([P, F], f16, tag=f"e{t}", name=f"e{t}")
                  for t, F in enumerate(plan)]

            first_m = [True]
            first_zm = [True]
            ycache = {}
            zcache = {}

            for ti, tok in enumerate(schedule):
                if "@" in tok:
                    q = tok[-1]
                    body = tok[: tok.index("@")]
                    if body[0] == "P":
                        half_sel = None
                        if body[1] in "ab":
                            half_sel, t = body[1], int(body[2:])
                        else:
                            t = int(body[1:])
                        F = plan[t]
                        lo, hi = 0, F
                        if half_sel == "a":
                            hi = F // 2
                        elif half_sel == "b":
                            lo = F // 2
                        sl = slice(offs[t] + lo, offs[t] + hi)
                        eng[q].dma_start(preds[t][:, lo:hi], predD[:, sl])
                    else:
                        half_sel = None
                        if body[1] in "ab":
                            half_sel, t = body[1], int(body[2:])
                        else:
                            t = int(body[1:])
                        F = plan[t]
                        lo, hi = 0, F
                        if half_sel == "a":
                            hi = F // 2
                        elif half_sel == "b":
                            lo = F // 2
                        sl = slice(offs[t] + lo, offs[t] + hi)
                        eng[q].dma_start(es[t][:, lo:hi], eD[:, sl])
                elif tok[0] == "Y":
                    t = int(tok[1:])
                    F = plan[t]
                    y_t = ypool.tile([P, F], f16, tag="y")
                    ye = nc.vector if y_eng[t] == "d" else nc.gpsimd
                    ye.tensor_tensor(y_t[:], preds[t][:], es[t][:], Alu.mult)
                    ycache[t] = y_t
                elif tok[0] == "T":
                    t = int(tok[1:])
                    F = plan[t]
                    jb = jpool.tile([P, F], f16, tag="jb")
                    nc.vector.tensor_scalar(
                        jb[:], es[t][:], 0.0, 0.0, Alu.max, Alu.add,
                        accum_out=acc[:, t : t + 1])
                elif tok[0] == "A":
                    t = int(tok[1:])
                    F = plan[t]
                    z_t = zpool.tile([P, F], f32r, tag="z")
                    nc.scalar.activation(z_t[:], ycache[t][:], Act.Ln,
                                         bias=half[:, 0:1], scale=1.0)
                    zcache[t] = z_t
                elif tok.startswith("ZM"):
                    t = int(tok[2:])
                    F = plan[t]
                    cw = F // 4
                    if first_zm[0]:
                        assert cw == 512, "first z-matmul must be full width"
                    for c in range(4):
                        nc.tensor.matmul(
                            psz[0:1, 512 * c : 512 * c + cw],
                            onesf[:, 0:1].bitcast(f32r),
                            zcache[t][:, c * cw : (c + 1) * cw],
                            start=first_zm[0], stop=(ti == last_zm),
                            skip_group_check=True)
                    first_zm[0] = False
                elif tok[0] == "M":
                    t = int(tok[1:])
                    F = plan[t]
                    cw = F // 4
                    if first_m[0]:
                        assert cw == 512, "first e-matmul must be full width"
                    for c in range(4):
                        nc.tensor.matmul(
                            pse[0:1, 512 * c : 512 * c + cw],
                            ones[:, 0:1], es[t][:, c * cw : (c + 1) * cw],
                            start=first_m[0], stop=(ti == last_m),
                            skip_group_check=True)
                    first_m[0] = False
                elif tok == "FE" or tok == "FZ":
                    ps = pse if tok == "FE" else psz
                    base = nt if tok == "FE" else nt + 4
                    for c in range(4):
                        on_act = (tok == "FZ" and c >= 2)
                        jf = jpool.tile([1, 512], f32,
                                        tag="jfa" if on_act else "jfd")
                        if on_act:
                            nc.scalar.activation(
                                jf[0:1, :], ps[0:1, 512 * c : 512 * c + 512],
                                Act.Copy,
                                accum_out=acc[0:1, base + c : base + c + 1])
                        else:
                            nc.vector.tensor_scalar(
                                jf[0:1, :], ps[0:1, 512 * c : 512 * c + 512],
                                0.0, 0.0, Alu.add, Alu.add,
                                accum_out=acc[0:1, base + c : base + c + 1])
                elif tok == "STATS":
                    nc.sync.dma_start(outD[:], acc[:])
                else:
                    raise ValueError(tok)
    nc.finalize()
    _NC_CACHE[key] = nc
    return nc


def _final_scalar(sw, sn, zsum, pred=None, gt=None, mask=None):
    """Host-side merge of per-core sums into the balance loss (f64)."""
    n_ignored = float(TOT) - (sw + sn)
    total_loss = -(zsum - n_ignored * _ln_half_bf16())  # pos + all-neg loss
    neg_count = min(sn, NEG_RATIO * sw)
    if neg_count >= sn:
        num = total_loss
    else:
        # exact OHEM fallback (not triggered for the shipped distribution)
        p = np.asarray(pred, dtype=np.float64).ravel()
        g = np.asarray(gt, dtype=np.float64).ravel()
        m = np.asarray(mask, dtype=np.float64).ravel()
        pos_loss = -(g * m * np.log(p)).sum()
        neg_loss = (1.0 - g) * m * (-np.log1p(-p))
        k = int(neg_count)
        if k <= 0:
            topk = 0.0
        else:
            part = np.partition(neg_loss, neg_loss.size - k)
            topk = float(part[neg_loss.size - k:].sum())
        num = pos_loss + topk
    if neg_count > 0:
        out = num / (sw + neg_count + EPS)
    else:
        out = num / (sw + EPS)
    return np.asarray(out, dtype=np.float32).reshape(())


def _encode(pred, gt, mask):
    # centered probabilities in fp16 (ulp 2^-12 near +-0.5), clamped one grid
    # point away from +-0.5 so Ln(y+1/2) never sees 0. Quantization cost on
    # the final loss measured at ~2.3e-4 rel (gate is 2e-2).
    lim = np.float32(0.5 - 2.0 ** -12)
    pc = np.clip(np.asarray(pred, dtype=np.float32) - np.float32(0.5),
                 -lim, lim)
    predf = np.ascontiguousarray(pc.astype(np.float16)).reshape(N_CORES, P, W)
    e = (np.asarray(mask, dtype=np.float32)
         * (2.0 * np.asarray(gt, dtype=np.float32) - 1.0))
    e = np.ascontiguousarray(e.astype(np.float16)).reshape(N_CORES, P, W)
    return predf, e


def run_device(pred, gt, mask, trace=False, **run_kwargs):
    _ensure_concourse()
    from concourse.bass_utils import run_bass_kernel_spmd

    nc = _build_nc()
    predf, e = _encode(pred, gt, mask)
    in_maps = [{"pred": predf[i], "e": e[i]} for i in range(N_CORES)]
    res = run_bass_kernel_spmd(nc, in_maps, list(range(N_CORES)), trace=trace,
                               **run_kwargs)
    stats = np.stack([np.asarray(r["stats"], dtype=np.float64)
                      for r in res.results])
    sw = stats[:, :, 0:NT].sum()
    se = stats[:, 0, NT:NT + 4].sum()
    zsum = stats[:, 0, NT + 4:NT + 8].sum()
    return (sw, sw - se, zsum), res


def kernel(pred, gt, mask):
    pred = np.asarray(pred, dtype=np.float32)
    gt = np.asarray(gt, dtype=np.float32)
    mask = np.asarray(mask, dtype=np.float32)
    if pred.shape != FULL_SHAPE:
        # defensive pure-host path for non-conforming shapes
        p64 = pred.astype(np.float64)
        sw = float((gt * mask).sum(dtype=np.float64))
        sn = float(((1.0 - gt) * mask).sum(dtype=np.float64))
        total = -(gt * mask * np.log(p64)
                  + (1.0 - gt) * mask * np.log1p(-p64)).sum()
        neg_count = min(sn, NEG_RATIO * sw)
        out = (total / (sw + neg_count + EPS) if neg_count > 0
               else total / (sw + EPS))
        return np.asarray(out, dtype=np.float32).reshape(())
    (se, sm, zsum), _ = run_device(pred, gt, mask)
    return _final_scalar(se, sm, zsum, pred, gt, mask)


# revision 6
# speedup vs baseline: 1.3731x; 1.1858x over previous
"""BalanceLoss (BCE + OHEM top-k negatives) on 8 trn2 NeuronCores — v2.

Algorithm
---------
Host encodes the two {0,1} label tensors as one categorical bf16 tensor
    e = mask * (2*gt - 1)  in {-1, 0, +1}   (pos / ignore / neg label)
Per core (data-parallel shard of 1/8 of the elements, [128 x 12800]):
    y = pred_centered * e                   (tensor_tensor, Pool/DVE;
                                             host ships pred - 1/2)
    z = Ln(y + 1/2)                         (ScalarE)
      = ln(pred)   where e=+1   (positive, masked-in)
      = ln(1-pred) where e=-1   (negative)
      = ln(1/2)    where e= 0   (masked-out; exact, host-corrected)
    z tiles are float32r so the PE column-sum runs at 1 cycle/row without
    bf16 rounding bias.
    Sz  = sum(z)       PE ones-matmul column sums into PSUM banks 0-3
    Se  = sum(e)       PE ones-matmul column sums into PSUM banks 4-7
    sw  = sum(relu(e)) DVE tensor_scalar(max 0) + accum (4x bf16 rate)
Host merge (exact f64): sn = sw - Se, Sm = sw + sn,
    pos_loss+neg_loss_all = -(Sz - (N-Sm)*ln(1/2)_bf16)
OHEM top-k == all-negatives whenever k = min(sn, 3*sw) == sn (true for this
distribution); exact host fallback otherwise.

Scheduling: all pred/e tiles stay resident in SBUF (no buffer recycling), and
the program is emitted from an explicit token sequence so each engine queue
(SP/Act/Pool DMA+compute, DVE, PE) receives jobs in a hand-tuned order that
keeps the ScalarE Ln stream — the longest serial chain — fed without stalls.
"""

import os
import sys

import numpy as np

# ---------------------------------------------------------------- constants
FULL_SHAPE = (32, 1, 640, 640)
TOT = 32 * 640 * 640          # 13_107_200 elements
N_CORES = 8
PER_CORE = TOT // N_CORES     # 1_638_400
P = 128                       # SBUF partitions
W = PER_CORE // P             # 12_800 free-dim elements per partition
NEG_RATIO = 3.0
EPS = 1e-6

TILE_PLAN = (128, 1024, 1408, 1792, 2048, 2048, 2048, 2048, 256)
assert sum(TILE_PLAN) == W
NT = len(TILE_PLAN)

# y-engine per tile: 'd' = DVE, 'g' = Pool
Y_ENG = ("d", "d", "d", "d", "g", "d", "d", "d", "d")

# Emission order. Tokens:
#   P<t>@<q>   pred-tile DMA on queue q (s/a/g); Pa/Pb = first/second half
#   E<t>@<q>   e-tile DMA
#   Y<t>       y stt (engine from Y_ENG)
#   T<t>       sum(relu(e)) = pos_count tensor_scalar on DVE
#   M<t>       PE e-matmuls for tile t
#   A<t>       Ln act on ScalarE
#   ZM<t>      PE z-matmuls for tile t (needs A<t> first)
#   FE / FZ    psum folds (e / z streams; split over Pool+DVE)
#   STATS      final stats DMA on SP
# PE stream init: M4/ZM4 are emitted first on PE with start=True (tile 4 is
# full-width 2048, so its write initializes all 512 cols of each psum group).
SCHEDULE = (
    "P0@s", "E0@a", "P1@g", "E1@a", "E2@s", "P2@s", "E3@g",
    "Y0", "T0", "A0",
    "P3@g",
    "Y1", "T1", "A1",
    "P4@s", "E4@g",
    "Y2", "T2", "A2",
    "P5@g", "E5@s", "M4", "M0", "M1",
    "Y3", "T3", "A3",
    "P6@s", "E6@g", "M2", "M3",
    "Y4", "T4", "A4", "ZM4", "ZM0", "ZM1", "ZM2", "ZM3",
    "P7@g", "E7@s", "M5",
    "P8@g", "E8@g", "M6", "M7", "M8",
    "Y5", "T5", "A5", "ZM5",
    "Y6", "T6", "A6", "ZM6",
    "Y8", "T8", "A8", "ZM8",
    "Y7", "T7", "A7", "ZM7",
    "FE", "FZ", "STATS",
)

_CONCOURSE_PATHS = ("/opt/trn_rl_repo", "/root/.axon_site/_ro/trn_rl_repo")


def _ensure_concourse():
    try:
        import concourse.bass  # noqa: F401
    except ImportError:
        for p in _CONCOURSE_PATHS:
            if os.path.isdir(p) and p not in sys.path:
                sys.path.insert(0, p)
        import concourse.bass  # noqa: F401


_NC_CACHE = {}

# ln(0.5) in fp32 — matches the device z value for masked-out elements
# (the Ln argument is exactly 0.5 there; z tiles are fp32(r)).
def _ln_half_bf16():
    return float(np.float32(np.log(np.float32(0.5))))


def _build_nc(plan=TILE_PLAN, y_eng=Y_ENG, schedule=SCHEDULE):
    key = (plan, y_eng, schedule)
    if key in _NC_CACHE:
        return _NC_CACHE[key]
    _ensure_concourse()
    import concourse.bacc as bacc
    import concourse.mybir as mybir
    import concourse.tile as tile

    f32 = mybir.dt.float32
    f32r = mybir.dt.float32r
    bf16 = mybir.dt.bfloat16
    f16 = mybir.dt.float16
    Act = mybir.ActivationFunctionType
    Alu = mybir.AluOpType

    nt = len(plan)
    offs = [sum(plan[:i]) for i in range(nt)]
    last_m = max(i for i, tok in enumerate(schedule) if tok.startswith("M"))
    last_zm = max(i for i, tok in enumerate(schedule) if tok.startswith("ZM"))
    # stats: [0:nt]=pos_count per tile, [nt]=Se fold, [nt+1]=Sz fold
    SCOLS = nt + 2

    nc = bacc.Bacc(None, target_bir_lowering=False)
    predD = nc.declare_dram_parameter("pred", [P, W], f16, isOutput=False)
    eD = nc.declare_dram_parameter("e", [P, W], f16, isOutput=False)
    outD = nc.declare_dram_parameter("stats", [P, SCOLS], f32, isOutput=True)

    with tile.TileContext(nc) as tc:
        eng = {"s": nc.sync, "a": nc.scalar, "g": nc.gpsimd}
        with (
            tc.tile_pool(name="iop", bufs=1) as iop,
            tc.tile_pool(name="ioe", bufs=1) as ioe,
            tc.tile_pool(name="ytmp", bufs=6) as ypool,
            tc.tile_pool(name="ztmp", bufs=9) as zpool,
            tc.tile_pool(name="junk", bufs=2) as jpool,
            tc.tile_pool(name="accp", bufs=1) as acc_pool,
            tc.tile_pool(name="ps", bufs=1, space="PSUM") as ps_pool,
        ):
            acc = acc_pool.tile([P, SCOLS], f32)
            nc.vector.memset(acc[:], 0.0)
            ones = acc_pool.tile([P, 1], f16)
            nc.gpsimd.memset(ones[:], 1.0)
            onesf = acc_pool.tile([P, 1], f32)
            nc.gpsimd.memset(onesf[:], 1.0)
            half = acc_pool.tile([P, 1], f32)
            nc.gpsimd.memset(half[:], 0.5)
            # 8 psum banks: z-stream groups 0-3 (cols 512c), e-stream 4-7.
            # The first-emitted matmul of each stream is a full-width tile
            # with start=True, initializing all 512 cols of each group.
            psz = ps_pool.tile([1, 512], f32, tag="psz")
            pse = ps_pool.tile([1, 512], f32, tag="pse")

            preds = [iop.tile([P, F], f16, tag=f"pred{t}", name=f"pred{t}")
                     for t, F in enumerate(plan)]
            es = [ioe.tile([P, F], f16, tag=f"e{t}", name=f"e{t}")
                  for t, F in enumerate(plan)]

            first_m = [True]
            first_zm = [True]
            ycache = {}
            zcache = {}

            for ti, tok in enumerate(schedule):
                if "@" in tok:
                    q = tok[-1]
                    body = tok[: tok.index("@")]
                    if body[0] == "P":
                        half_sel = None
                        if body[1] in "ab":
                            half_sel, t = body[1], int(body[2:])
                        else:
                            t = int(body[1:])
                        F = plan[t]
                        lo, hi = 0, F
                        if half_sel == "a":
                            hi = F // 2
                        elif half_sel == "b":
                            lo = F // 2
                        sl = slice(offs[t] + lo, offs[t] + hi)
                        eng[q].dma_start(preds[t][:, lo:hi], predD[:, sl])
                    else:
                        half_sel = None
                        if body[1] in "ab":
                            half_sel, t = body[1], int(body[2:])
                        else:
                            t = int(body[1:])
                        F = plan[t]
                        lo, hi = 0, F
                        if half_sel == "a":
                            hi = F // 2
                        elif half_sel == "b":
                            lo = F // 2
                        sl = slice(offs[t] + lo, offs[t] + hi)
                        eng[q].dma_start(es[t][:, lo:hi], eD[:, sl])
                elif tok[0] == "Y":
                    t = int(tok[1:])
                    F = plan[t]
                    y_t = ypool.tile([P, F], f16, tag="y")
                    ye = nc.vector if y_eng[t] == "d" else nc.gpsimd
                    ye.tensor_tensor(y_t[:], preds[t][:], es[t][:], Alu.mult)
                    ycache[t] = y_t
                elif tok[0] == "T":
                    t = int(tok[1:])
                    F = plan[t]
                    jb = jpool.tile([P, F], f16, tag="jb")
                    nc.vector.tensor_scalar(
                        jb[:], es[t][:], 0.0, 0.0, Alu.max, Alu.add,
                        accum_out=acc[:, t : t + 1])
                elif tok[0] == "A":
                    t = int(tok[1:])
                    F = plan[t]
                    z_t = zpool.tile([P, F], f32r, tag="z")
                    nc.scalar.activation(z_t[:], ycache[t][:], Act.Ln,
                                         bias=half[:, 0:1], scale=1.0)
                    zcache[t] = z_t
                elif tok.startswith("ZM"):
                    t = int(tok[2:])
                    F = plan[t]
                    cw = F // 4
                    if first_zm[0]:
                        assert cw == 512, "first z-matmul must be full width"
                    for c in range(4):
                        nc.tensor.matmul(
                            psz[0:1, 0:cw],
                            onesf[:, 0:1].bitcast(f32r),
                            zcache[t][:, c * cw : (c + 1) * cw],
                            start=(first_zm[0] and c == 0),
                            stop=(ti == last_zm and c == 3),
                            skip_group_check=True)
                    first_zm[0] = False
                elif tok[0] == "M":
                    t = int(tok[1:])
                    F = plan[t]
                    cw = F // 4
                    if first_m[0]:
                        assert cw == 512, "first e-matmul must be full width"
                    for c in range(4):
                        nc.tensor.matmul(
                            pse[0:1, 0:cw],
                            ones[:, 0:1], es[t][:, c * cw : (c + 1) * cw],
                            start=(first_m[0] and c == 0),
                            stop=(ti == last_m and c == 3),
                            skip_group_check=True)
                    first_m[0] = False
                elif tok == "FE" or tok == "FZ":
                    ps = pse if tok == "FE" else psz
                    base = nt if tok == "FE" else nt + 1
                    jf = jpool.tile([1, 512], f32, tag="jfd")
                    nc.vector.tensor_scalar(
                        jf[0:1, :], ps[0:1, 0:512],
                        0.0, 0.0, Alu.add, Alu.add,
                        accum_out=acc[0:1, base : base + 1])
                elif tok == "STATS":
                    nc.sync.dma_start(outD[:], acc[:])
                else:
                    raise ValueError(tok)
    nc.finalize()
    _NC_CACHE[key] = nc
    return nc


def _final_scalar(sw, sn, zsum, pred=None, gt=None, mask=None):
    """Host-side merge of per-core sums into the balance loss (f64)."""
    n_ignored = float(TOT) - (sw + sn)
    total_loss = -(zsum - n_ignored * _ln_half_bf16())  # pos + all-neg loss
    neg_count = min(sn, NEG_RATIO * sw)
    if neg_count >= sn:
        num = total_loss
    else:
        # exact OHEM fallback (not triggered for the shipped distribution)
        p = np.asarray(pred, dtype=np.float64).ravel()
        g = np.asarray(gt, dtype=np.float64).ravel()
        m = np.asarray(mask, dtype=np.float64).ravel()
        pos_loss = -(g * m * np.log(p)).sum()
        neg_loss = (1.0 - g) * m * (-np.log1p(-p))
        k = int(neg_count)
        if k <= 0:
            topk = 0.0
        else:
            part = np.partition(neg_loss, neg_loss.size - k)
            topk = float(part[neg_loss.size - k:].sum())
        num = pos_loss + topk
    if neg_count > 0:
        out = num / (sw + neg_count + EPS)
    else:
        out = num / (sw + EPS)
    return np.asarray(out, dtype=np.float32).reshape(())


def _encode(pred, gt, mask):
    # centered probabilities in fp16 (ulp 2^-12 near +-0.5), clamped one grid
    # point away from +-0.5 so Ln(y+1/2) never sees 0. Quantization cost on
    # the final loss measured at ~2.3e-4 rel (gate is 2e-2).
    lim = np.float32(0.5 - 2.0 ** -12)
    pc = np.clip(np.asarray(pred, dtype=np.float32) - np.float32(0.5),
                 -lim, lim)
    predf = np.ascontiguousarray(pc.astype(np.float16)).reshape(N_CORES, P, W)
    e = (np.asarray(mask, dtype=np.float32)
         * (2.0 * np.asarray(gt, dtype=np.float32) - 1.0))
    e = np.ascontiguousarray(e.astype(np.float16)).reshape(N_CORES, P, W)
    return predf, e


def run_device(pred, gt, mask, trace=False, **run_kwargs):
    _ensure_concourse()
    from concourse.bass_utils import run_bass_kernel_spmd

    nc = _build_nc()
    predf, e = _encode(pred, gt, mask)
    in_maps = [{"pred": predf[i], "e": e[i]} for i in range(N_CORES)]
    res = run_bass_kernel_spmd(nc, in_maps, list(range(N_CORES)), trace=trace,
                               **run_kwargs)
    stats = np.stack([np.asarray(r["stats"], dtype=np.float64)
                      for r in res.results])
    sw = stats[:, :, 0:NT].sum()
    se = stats[:, 0, NT].sum()
    zsum = stats[:, 0, NT + 1].sum()
    return (sw, sw - se, zsum), res


def kernel(pred, gt, mask):
    pred = np.asarray(pred, dtype=np.float32)
    gt = np.asarray(gt, dtype=np.float32)
    mask = np.asarray(mask, dtype=np.float32)
    if pred.shape != FULL_SHAPE:
        # defensive pure-host path for non-conforming shapes
        p64 = pred.astype(np.float64)
        sw = float((gt * mask).sum(dtype=np.float64))
        sn = float(((1.0 - gt) * mask).sum(dtype=np.float64))
        total = -(gt * mask * np.log(p64)
                  + (1.0 - gt) * mask * np.log1p(-p64)).sum()
        neg_count = min(sn, NEG_RATIO * sw)
        out = (total / (sw + neg_count + EPS) if neg_count > 0
               else total / (sw + EPS))
        return np.asarray(out, dtype=np.float32).reshape(())
    (se, sm, zsum), _ = run_device(pred, gt, mask)
    return _final_scalar(se, sm, zsum, pred, gt, mask)
